# revision 7
# baseline (speedup 1.0000x reference)
"""Distributed Trainium2 kernel for nn_Attention (causal MHA with direct-reshape
head view).

Reference semantics (B=2, S=2048, D=1024, H=16, hd=64):
    qp = hs @ Wq.T  -> [B, S, D], then q = qp.reshape(B, H, S, hd)  (DIRECT view:
    head h's token t = 16*rr + j has features qp[b, 128*h + rr, 64*j : 64*j+64])
    k, v likewise; causal attention in t with softmax(wei / hd);
    ctx -> [B, S2, H, hd] -> reshape [B, S, D]; out = ctx @ Wo.T + bo.

Sharding (uniform SPMD across 8 cores, one AllToAll):
  - Head h consumes only hs rows 128h..128h+128 (per batch). Core c owns heads
    2c, 2c+1 => hs rows 256c..256c+256 of each batch (512 rows total, disjoint
    across cores). Host passes hsT_sh = those rows, transposed [1024, 512].
  - Core projects its rows against full Wq/Wk/Wv (normal layout, to DRAM),
    then re-reads through the reshape view: token-tiles [128 t, 64 d] are flat
    8-row slices. K/Q are PE-transposed to [64 d, t] layout; V is used as-is.
  - Causal flash attention per (batch, head) in t-space; denominator rides a
    ones-column appended to V; normalization via K=1 broadcast matmul.
  - Normalized ctx^T shards [8 q-blocks, 128 dims, 512 q] go through one
    AllToAll; core c ends with all 1024 ctx dims for global query block c and
    output-projects it (+bias). Host concatenates the 8 [512, 1024] blocks.

All matmuls bf16 with fp32 PSUM accumulation; softmax needs no max-subtraction
(logits/64 are tiny).
"""

import sys

for _p in ("/opt/trn_rl_repo", "/opt/pypackages"):
    if _p not in sys.path:
        sys.path.append(_p)

import numpy as np
import ml_dtypes

import concourse.bass as bass
import concourse.mybir as mybir
import concourse.tile as tile
from concourse import bacc
from concourse.bass_utils import run_bass_kernel_spmd

N_CORES = 8
B, S, D, H = 2, 2048, 1024, 16
HD = D // H          # 64 head dim
NT = B * S           # 4096 tokens
QBLK = NT // N_CORES  # 512 query rows per output block
NROWS = 512          # hs rows owned per core (2 heads x 2 batches x 128)

BF16 = mybir.dt.bfloat16
F32 = mybir.dt.float32
F32R = mybir.dt.float32r

_cached = {}


def build_nc():
    nc = bacc.Bacc("TRN2", target_bir_lowering=False, debug=False,
                   num_devices=N_CORES)

    # ---- I/O -----------------------------------------------------------
    hsT_sh = nc.dram_tensor("hsT_sh", [D, NROWS], BF16, kind="ExternalInput")
    wqT = nc.dram_tensor("wqT", [D, D], BF16, kind="ExternalInput")
    wkT = nc.dram_tensor("wkT", [D, D], BF16, kind="ExternalInput")
    wvT = nc.dram_tensor("wvT", [D, D], BF16, kind="ExternalInput")
    woT = nc.dram_tensor("woT", [D, D], BF16, kind="ExternalInput")
    bo_rep = nc.dram_tensor("bo_rep", [128, D], F32, kind="ExternalInput")
    masks = nc.dram_tensor("masks", [128, 1536], BF16, kind="ExternalInput")
    onesr = nc.dram_tensor("onesr", [128, 64], F32R, kind="ExternalInput")
    ident = nc.dram_tensor("ident", [128, 128], BF16, kind="ExternalInput")
    out_ext = nc.dram_tensor("out", [QBLK, D], F32, kind="ExternalOutput")

    # internal DRAM
    qn = nc.dram_tensor("qn", [NROWS, D], BF16)   # projections, normal layout
    kn = nc.dram_tensor("kn", [NROWS, D], BF16)
    vn = nc.dram_tensor("vn", [NROWS, D], BF16)
    ctxn_sh = nc.dram_tensor("ctxn_sh", [N_CORES, 128, QBLK], BF16)
    ctxn_a2a = nc.dram_tensor("ctxn_a2a", [N_CORES, 128, QBLK], BF16)

    EXP = mybir.ActivationFunctionType.Exp

    with tile.TileContext(nc) as tc:
        with (
            tc.tile_pool(name="persist", bufs=1) as persist,
            tc.tile_pool(name="wtile", bufs=3) as w_pool,
            tc.tile_pool(name="evict", bufs=4) as evict_pool,
        ):
            ident_sb = persist.tile([128, 128], BF16)
            masks_sb = persist.tile([128, 1536], BF16)
            ones_sb = persist.tile([128, 64], F32R)
            hs_sb = persist.tile([128, 8, NROWS], BF16)   # hsT_sh resident
            kt_sb = persist.tile([128, 2, 2048], BF16)    # K^T per batch (A|B rows)
            qt_sb = persist.tile([128, 2, 2048], BF16)    # Q^T per batch
            vaug_sb = persist.tile([128, 4, 16, 65], BF16)  # V tiles + ones col

            nc.sync.dma_start(out=ident_sb[:], in_=ident[:])
            nc.sync.dma_start(out=masks_sb[:], in_=masks[:])
            nc.sync.dma_start(out=ones_sb[:], in_=onesr[:])
            for dt_ in range(8):
                nc.sync.dma_start(out=hs_sb[:, dt_, :],
                                  in_=hsT_sh[dt_ * 128:(dt_ + 1) * 128, :])
            nc.vector.memset(vaug_sb[:], 1.0)

            # ============ Phase A: Q/K/V projections (normal layout) ============
            with tc.tile_pool(name="pa_psum", bufs=4, space="PSUM") as pa_psum:
                for w_ext, dest in ((wqT, qn), (wkT, kn), (wvT, vn)):
                    for dh in range(2):  # dout halves of 512
                        ps = [pa_psum.tile([128, 512], F32, name="pp") for _ in range(4)]
                        for dt_ in range(8):
                            w_t = w_pool.tile([128, 512], BF16, name="w_t")
                            nc.sync.dma_start(
                                out=w_t[:],
                                in_=w_ext[dt_ * 128:(dt_ + 1) * 128,
                                          dh * 512:(dh + 1) * 512])
                            for rb in range(4):
                                nc.tensor.matmul(
                                    ps[rb], lhsT=hs_sb[:, dt_, rb * 128:(rb + 1) * 128],
                                    rhs=w_t[:], start=(dt_ == 0), stop=(dt_ == 7))
                        for rb in range(4):
                            ev = evict_pool.tile([128, 512], BF16, name="ev")
                            nc.scalar.copy(ev[:], ps[rb][:])
                            nc.sync.dma_start(
                                out=dest[rb * 128:(rb + 1) * 128,
                                         dh * 512:(dh + 1) * 512],
                                in_=ev[:])

            # ===== Phase B: reshape-view readback; build K^T, Q^T, V-aug =====
            with (
                tc.tile_pool(name="tr_in", bufs=4) as tr_pool,
                tc.tile_pool(name="tr_psum", bufs=2, space="PSUM") as tr_psum,
            ):
                for b in range(B):
                    for d_ in range(2):  # head delta within pair
                        rowbase = 256 * b + 128 * d_
                        for m in range(16):  # token tiles of 128 (8 rows each)
                            nc.sync.dma_start(
                                out=vaug_sb[:, 2 * b + d_, m, 0:64],
                                in_=vn[rowbase + 8 * m: rowbase + 8 * (m + 1), :])
                        for g in range(4):  # groups of 4 token-tiles
                            pk = tr_psum.tile([128, 512], BF16, name="pk")
                            pq = tr_psum.tile([128, 512], BF16, name="pq")
                            for i in range(4):
                                m = 4 * g + i
                                ktile = tr_pool.tile([128, 64], BF16, name="ktile")
                                qtile = tr_pool.tile([128, 64], BF16, name="qtile")
                                nc.sync.dma_start(
                                    out=ktile[:],
                                    in_=kn[rowbase + 8 * m: rowbase + 8 * (m + 1), :])
                                nc.sync.dma_start(
                                    out=qtile[:],
                                    in_=qn[rowbase + 8 * m: rowbase + 8 * (m + 1), :])
                                tp = (0, 64 * d_)
                                nc.tensor.transpose(
                                    pk[64 * d_:64 * d_ + 64, i * 128:(i + 1) * 128],
                                    ktile[:], ident_sb[:], tile_position=tp)
                                nc.tensor.transpose(
                                    pq[64 * d_:64 * d_ + 64, i * 128:(i + 1) * 128],
                                    qtile[:], ident_sb[:], tile_position=tp)
                            sl = slice(64 * d_, 64 * d_ + 64)
                            nc.scalar.copy(kt_sb[sl, b, g * 512:(g + 1) * 512],
                                           pk[sl, :])
                            nc.scalar.copy(qt_sb[sl, b, g * 512:(g + 1) * 512],
                                           pq[sl, :])

            # ================= Phase D: attention (own 2 heads) =================
            with (
                tc.tile_pool(name="s_psum", bufs=2, space="PSUM") as s_psum,
                tc.tile_pool(name="ctx_psum", bufs=1, space="PSUM") as ctx_psum,
                tc.tile_pool(name="bc_psum", bufs=1, space="PSUM") as bc_psum,
                tc.tile_pool(name="p_sbuf", bufs=4) as p_pool,
                tc.tile_pool(name="rd_sbuf", bufs=4) as rd_pool,
                tc.tile_pool(name="ctxn_sbuf", bufs=4) as ctxn_pool,
            ):
                for b in range(B):
                    for J in range(4):  # 512-query tiles (t-space)
                        qb = 4 * b + J              # global query block id
                        nt_full = 4 * J + 2         # full-width k-tiles
                        ctxA = ctx_psum.tile([65, 512], F32, name="ctxA")
                        ctxB = ctx_psum.tile([65, 512], F32, name="ctxB")
                        # -- full-width k-tiles --
                        for t in range(nt_full):
                            kt = kt_sb[:, b, 128 * t:128 * (t + 1)]
                            sf = s_psum.tile([128, 1024], F32, name="sh")
                            nc.tensor.matmul(sf[:, 0:512], lhsT=kt[0:64, :],
                                             rhs=qt_sb[0:64, b, 512 * J:512 * (J + 1)],
                                             tile_position=(0, 0))
                            nc.tensor.matmul(sf[:, 512:1024], lhsT=kt[64:128, :],
                                             rhs=qt_sb[64:128, b, 512 * J:512 * (J + 1)],
                                             tile_position=(64, 0))
                            pf = p_pool.tile([128, 1024], BF16, name="ph")
                            nc.scalar.activation(pf[:], sf[:], EXP, scale=1.0 / HD)
                            if t >= nt_full - 2:
                                # causal mask on q-half 0 for both heads
                                mo = 0 if t == nt_full - 2 else 256
                                nc.vector.tensor_mul(pf[:, 0:256], pf[:, 0:256],
                                                     masks_sb[:, mo:mo + 256])
                                nc.vector.tensor_mul(pf[:, 512:768], pf[:, 512:768],
                                                     masks_sb[:, mo:mo + 256])
                            nc.tensor.matmul(ctxA[:], lhsT=vaug_sb[:, 2 * b, t, :],
                                             rhs=pf[:, 0:512], start=(t == 0),
                                             stop=False, skip_group_check=True)
                            nc.tensor.matmul(ctxB[:], lhsT=vaug_sb[:, 2 * b + 1, t, :],
                                             rhs=pf[:, 512:1024], start=(t == 0),
                                             stop=False, skip_group_check=True)
                        # -- two diagonal half-tiles last (q-half 1 only) --
                        for m in range(2):
                            t = nt_full + m
                            kt = kt_sb[:, b, 128 * t:128 * (t + 1)]
                            # A in bank 0 (cols 0:256), B in bank 1 (cols 512:768):
                            # same-bank row-packed matmul pairs crash the device.
                            sh = s_psum.tile([128, 1024], F32, name="sh")
                            nc.tensor.matmul(sh[:, 0:256], lhsT=kt[0:64, :],
                                             rhs=qt_sb[0:64, b, 512 * J + 256:512 * (J + 1)],
                                             tile_position=(0, 0))
                            nc.tensor.matmul(sh[:, 512:768], lhsT=kt[64:128, :],
                                             rhs=qt_sb[64:128, b, 512 * J + 256:512 * (J + 1)],
                                             tile_position=(64, 0))
                            ph = p_pool.tile([128, 1024], BF16, name="ph")
                            moff = 512 + 512 * m  # [M0|M0] then [M1|M1]
                            nc.scalar.activation(ph[:, 0:256], sh[:, 0:256], EXP,
                                                 scale=1.0 / HD)
                            nc.scalar.activation(ph[:, 512:768], sh[:, 512:768], EXP,
                                                 scale=1.0 / HD)
                            nc.vector.tensor_mul(ph[:, 0:256], ph[:, 0:256],
                                                 masks_sb[:, moff:moff + 256])
                            nc.vector.tensor_mul(ph[:, 512:768], ph[:, 512:768],
                                                 masks_sb[:, moff:moff + 256])
                            nc.tensor.matmul(ctxA[:, 256:512],
                                             lhsT=vaug_sb[:, 2 * b, t, :],
                                             rhs=ph[:, 0:256], start=False,
                                             stop=(m == 1), skip_group_check=True)
                            nc.tensor.matmul(ctxB[:, 256:512],
                                             lhsT=vaug_sb[:, 2 * b + 1, t, :],
                                             rhs=ph[:, 512:768], start=False,
                                             stop=(m == 1), skip_group_check=True)
                        # -- normalize + evict --
                        rdA = rd_pool.tile([65, 512], F32R, name="rdA")
                        rdB = rd_pool.tile([65, 512], F32R, name="rdB")
                        with nc.allow_low_precision("f32r is full fp32 storage"):
                            nc.vector.reciprocal(rdA[64:65, :], ctxA[64:65, :])
                            nc.vector.reciprocal(rdB[64:65, :], ctxB[64:65, :])
                        bcA = bc_psum.tile([64, 512], F32, name="bcA")
                        bcB = bc_psum.tile([64, 512], F32, name="bcB")
                        nc.tensor.matmul(bcA, lhsT=ones_sb[64:65, :], rhs=rdA[64:65, :],
                                         tile_position=(64, 0))
                        nc.tensor.matmul(bcB, lhsT=ones_sb[64:65, :], rhs=rdB[64:65, :],
                                         tile_position=(64, 0))
                        bcA_sb = rd_pool.tile([64, 512], F32, name="bcA_sb")
                        bcB_sb = rd_pool.tile([64, 512], F32, name="bcB_sb")
                        nc.vector.tensor_copy(bcA_sb, bcA[:])
                        nc.vector.tensor_copy(bcB_sb, bcB[:])
                        cnA = ctxn_pool.tile([64, 512], BF16, name="cnA")
                        cnB = ctxn_pool.tile([64, 512], BF16, name="cnB")
                        nc.vector.tensor_mul(cnA, ctxA[0:64, :], bcA_sb[:])
                        nc.vector.tensor_mul(cnB, ctxB[0:64, :], bcB_sb[:])
                        nc.sync.dma_start(out=ctxn_sh[qb, 0:64, :], in_=cnA[:])
                        nc.sync.dma_start(out=ctxn_sh[qb, 64:128, :], in_=cnB[:])

            # ================= AllToAll =================
            nc.gpsimd.collective_compute(
                "AllToAll",
                mybir.AluOpType.bypass,
                replica_groups=[list(range(N_CORES))],
                ins=[ctxn_sh[:].opt()],
                outs=[ctxn_a2a[:].opt()],
            )

            # ================= Phase E: output projection =================
            with (
                tc.tile_pool(name="pe_psum", bufs=4, space="PSUM") as pe_psum,
                tc.tile_pool(name="pe_sbuf", bufs=2) as pe_pool,
            ):
                ea_sb = persist.tile([128, 8, 512], BF16)
                wo_sb = persist.tile([128, 8, 1024], BF16)
                bo_sb = persist.tile([128, 1024], F32)
                nc.sync.dma_start(out=bo_sb[:], in_=bo_rep[:])
                for r in range(8):
                    nc.sync.dma_start(out=ea_sb[:, r, :], in_=ctxn_a2a[r, :, :])
                    nc.sync.dma_start(out=wo_sb[:, r, :], in_=woT[r * 128:(r + 1) * 128, :])
                for qs in range(4):
                    for dh in range(2):
                        psum_o = pe_psum.tile([128, 512], F32, name="psum_o")
                        for r in range(8):
                            nc.tensor.matmul(psum_o,
                                             lhsT=ea_sb[:, r, qs * 128:(qs + 1) * 128],
                                             rhs=wo_sb[:, r, dh * 512:(dh + 1) * 512],
                                             start=(r == 0), stop=(r == 7))
                        ot = pe_pool.tile([128, 512], F32, name="ot")
                        nc.vector.tensor_add(ot, psum_o[:], bo_sb[:, dh * 512:(dh + 1) * 512])
                        nc.sync.dma_start(
                            out=out_ext[qs * 128:(qs + 1) * 128, dh * 512:(dh + 1) * 512],
                            in_=ot[:])

    nc.compile()
    return nc


def _prep_inputs(hidden_states, Wq, Wk, Wv, Wo, bo):
    bf = ml_dtypes.bfloat16
    hs = np.asarray(hidden_states, dtype=np.float32).reshape(NT, D)
    WqT = np.ascontiguousarray(np.asarray(Wq, np.float32).T).astype(bf)
    WkT = np.ascontiguousarray(np.asarray(Wk, np.float32).T).astype(bf)
    WvT = np.ascontiguousarray(np.asarray(Wv, np.float32).T).astype(bf)
    WoT = np.ascontiguousarray(np.asarray(Wo, np.float32).T).astype(bf)
    bo_rep = np.tile(np.asarray(bo, np.float32)[None, :], (128, 1))

    p = np.arange(128)[:, None]
    f = np.arange(256)[None, :]
    M0 = (p <= f).astype(np.float32)
    M1 = (p + 128 <= f).astype(np.float32)
    masks = np.concatenate([M0, M1, M0, M0, M1, M1], axis=1).astype(bf)  # [128,1536]
    onesr = np.ones((128, 64), dtype=np.float32)
    ident = np.eye(128, dtype=np.float32).astype(bf)

    in_maps = []
    for c in range(N_CORES):
        rows = np.concatenate([np.arange(256) + b * 2048 + 256 * c for b in range(B)])
        hsT_sh = np.ascontiguousarray(hs[rows].T).astype(bf)  # [1024, 512]
        in_maps.append({
            "hsT_sh": hsT_sh,
            "wqT": WqT, "wkT": WkT, "wvT": WvT, "woT": WoT,
            "bo_rep": bo_rep, "masks": masks, "onesr": onesr, "ident": ident,
        })
    return in_maps


def kernel(hidden_states, Wq, Wk, Wv, Wo, bo, _trace=False, _trace_kwargs=None):
    if "nc" not in _cached:
        _cached["nc"] = build_nc()
    nc = _cached["nc"]
    in_maps = _prep_inputs(hidden_states, Wq, Wk, Wv, Wo, bo)
    res = run_bass_kernel_spmd(nc, in_maps, core_ids=list(range(N_CORES)),
                               trace=_trace, **(_trace_kwargs or {}))
    _cached["last_result"] = res
    out = np.concatenate([res.results[c]["out"] for c in range(N_CORES)], axis=0)
    return out.reshape(B, S, D).astype(np.float32)


# revision 10
# speedup vs baseline: 1.2660x; 1.2660x over previous
"""Distributed Trainium2 kernel for nn_Attention (causal MHA with direct-reshape
head view).

Reference semantics (B=2, S=2048, D=1024, H=16, hd=64):
    qp = hs @ Wq.T  -> [B, S, D], then q = qp.reshape(B, H, S, hd)  (DIRECT view:
    head h's token t = 16*rr + j has features qp[b, 128*h + rr, 64*j : 64*j+64])
    k, v likewise; causal attention in t with softmax(wei / hd);
    ctx -> [B, S2, H, hd] -> reshape [B, S, D]; out = ctx @ Wo.T + bo.

Sharding (uniform SPMD across 8 cores, one AllToAll):
  - Head h consumes only hs rows 128h..128h+128 (per batch). Core c owns heads
    2c, 2c+1 => hs rows 256c..256c+256 of each batch (512 rows total, disjoint
    across cores). Host passes hsT_sh = those rows, transposed [1024, 512].
  - Core projects its rows against full Wq/Wk/Wv (normal layout, to DRAM),
    then re-reads through the reshape view: token-tiles [128 t, 64 d] are flat
    8-row slices. K/Q are PE-transposed to [64 d, t] layout; V is used as-is.
  - Causal flash attention per (batch, head) in t-space; denominator rides a
    ones-column appended to V; normalization via K=1 broadcast matmul.
  - Normalized ctx^T shards [8 q-blocks, 128 dims, 512 q] go through one
    AllToAll; core c ends with all 1024 ctx dims for global query block c and
    output-projects it (+bias). Host concatenates the 8 [512, 1024] blocks.

All matmuls bf16 with fp32 PSUM accumulation; softmax needs no max-subtraction
(logits/64 are tiny).
"""

import sys

for _p in ("/opt/trn_rl_repo", "/opt/pypackages"):
    if _p not in sys.path:
        sys.path.append(_p)

import numpy as np
import ml_dtypes

import concourse.bass as bass
import concourse.mybir as mybir
import concourse.tile as tile
from concourse import bacc
from concourse.bass_utils import run_bass_kernel_spmd

N_CORES = 8
B, S, D, H = 2, 2048, 1024, 16
HD = D // H          # 64 head dim
NT = B * S           # 4096 tokens
QBLK = NT // N_CORES  # 512 query rows per output block
NROWS = 512          # hs rows owned per core (2 heads x 2 batches x 128)

BF16 = mybir.dt.bfloat16
F32 = mybir.dt.float32
F32R = mybir.dt.float32r

_cached = {}


def build_nc():
    nc = bacc.Bacc("TRN2", target_bir_lowering=False, debug=False,
                   num_devices=N_CORES)

    # ---- I/O -----------------------------------------------------------
    hsT_sh = nc.dram_tensor("hsT_sh", [D, NROWS], BF16, kind="ExternalInput")
    wqT = nc.dram_tensor("wqT", [D, D], BF16, kind="ExternalInput")
    wkT = nc.dram_tensor("wkT", [D, D], BF16, kind="ExternalInput")
    wvT = nc.dram_tensor("wvT", [D, D], BF16, kind="ExternalInput")
    woT = nc.dram_tensor("woT", [D, D], BF16, kind="ExternalInput")
    bo_rep = nc.dram_tensor("bo_rep", [128, D], F32, kind="ExternalInput")
    masks = nc.dram_tensor("masks", [128, 1536], BF16, kind="ExternalInput")
    onesr = nc.dram_tensor("onesr", [128, 64], F32, kind="ExternalInput")
    ident = nc.dram_tensor("ident", [128, 128], BF16, kind="ExternalInput")
    out_ext = nc.dram_tensor("out", [QBLK, D], F32, kind="ExternalOutput")

    # internal DRAM
    qn = nc.dram_tensor("qn", [NROWS, D], BF16)   # projections, normal layout
    kn = nc.dram_tensor("kn", [NROWS, D], BF16)
    vn = nc.dram_tensor("vn", [NROWS, D], BF16)
    ctxn_sh = nc.dram_tensor("ctxn_sh", [N_CORES, 128, QBLK], BF16)
    ctxn_a2a = nc.dram_tensor("ctxn_a2a", [N_CORES, 128, QBLK], BF16)

    EXP = mybir.ActivationFunctionType.Exp

    with tile.TileContext(nc) as tc:
        with (
            tc.tile_pool(name="persist", bufs=1) as persist,
            tc.tile_pool(name="wtile", bufs=3) as w_pool,
            tc.tile_pool(name="evict", bufs=4) as evict_pool,
        ):
            ident_sb = persist.tile([128, 128], BF16)
            masks_sb = persist.tile([128, 1536], BF16)
            ones_sb = persist.tile([128, 64], F32)
            hs_sb = persist.tile([128, 8, NROWS], BF16)   # hsT_sh resident
            kt_sb = persist.tile([128, 2, 2048], BF16)    # K^T per batch (A|B rows)
            qt_sb = persist.tile([128, 2, 2048], BF16)    # Q^T per batch
            vaug_sb = persist.tile([128, 4, 16, 65], BF16)  # V tiles + ones col

            nc.sync.dma_start(out=ident_sb[:], in_=ident[:])
            nc.sync.dma_start(out=masks_sb[:], in_=masks[:])
            nc.sync.dma_start(out=ones_sb[:], in_=onesr[:])
            for dt_ in range(8):
                nc.gpsimd.dma_start(out=hs_sb[:, dt_, :],
                                    in_=hsT_sh[dt_ * 128:(dt_ + 1) * 128, :])
            nc.vector.memset(vaug_sb[:], 1.0)

            # ============ Phase A: Q/K/V projections (normal layout) ============
            with tc.tile_pool(name="pa_psum", bufs=4, space="PSUM") as pa_psum:
                for w_ext, dest in ((wqT, qn), (wkT, kn), (wvT, vn)):
                    for dh in range(2):  # dout halves of 512
                        ps = [pa_psum.tile([128, 512], F32, name="pp") for _ in range(4)]
                        for dt_ in range(8):
                            w_t = w_pool.tile([128, 512], BF16, name="w_t")
                            nc.gpsimd.dma_start(
                                out=w_t[:],
                                in_=w_ext[dt_ * 128:(dt_ + 1) * 128,
                                          dh * 512:(dh + 1) * 512])
                            for rb in range(4):
                                nc.tensor.matmul(
                                    ps[rb], lhsT=hs_sb[:, dt_, rb * 128:(rb + 1) * 128],
                                    rhs=w_t[:], start=(dt_ == 0), stop=(dt_ == 7))
                        for rb in range(4):
                            ev = evict_pool.tile([128, 512], BF16, name="ev")
                            nc.scalar.copy(ev[:], ps[rb][:])
                            nc.gpsimd.dma_start(
                                out=dest[rb * 128:(rb + 1) * 128,
                                         dh * 512:(dh + 1) * 512],
                                in_=ev[:])

            # ===== Phase B: reshape-view readback; build K^T, Q^T, V-aug =====
            with (
                tc.tile_pool(name="tr_in", bufs=4) as tr_pool,
                tc.tile_pool(name="tr_psum", bufs=2, space="PSUM") as tr_psum,
            ):
                RSH = "(a s) (j d) -> (s j) a d"
                for b in range(B):
                    for d_ in range(2):  # head delta within pair
                        rowbase = 256 * b + 128 * d_
                        nc.sync.dma_start(
                            out=vaug_sb[:, 2 * b + d_, :, 0:64],
                            in_=vn[rowbase:rowbase + 128, :].rearrange(RSH, a=16, j=16))
                        kb = tr_pool.tile([128, 16, 64], BF16, name="ktile")
                        qb_t = tr_pool.tile([128, 16, 64], BF16, name="qtile")
                        nc.sync.dma_start(
                            out=kb[:],
                            in_=kn[rowbase:rowbase + 128, :].rearrange(RSH, a=16, j=16))
                        nc.sync.dma_start(
                            out=qb_t[:],
                            in_=qn[rowbase:rowbase + 128, :].rearrange(RSH, a=16, j=16))
                        for g in range(4):  # groups of 4 token-tiles
                            pk = tr_psum.tile([128, 512], BF16, name="pk")
                            pq = tr_psum.tile([128, 512], BF16, name="pq")
                            for i in range(4):
                                m = 4 * g + i
                                tp = (0, 64 * d_)
                                nc.tensor.transpose(
                                    pk[64 * d_:64 * d_ + 64, i * 128:(i + 1) * 128],
                                    kb[:, m, :], ident_sb[:], tile_position=tp)
                                nc.tensor.transpose(
                                    pq[64 * d_:64 * d_ + 64, i * 128:(i + 1) * 128],
                                    qb_t[:, m, :], ident_sb[:], tile_position=tp)
                            sl = slice(64 * d_, 64 * d_ + 64)
                            nc.scalar.copy(kt_sb[sl, b, g * 512:(g + 1) * 512],
                                           pk[sl, :])
                            nc.scalar.copy(qt_sb[sl, b, g * 512:(g + 1) * 512],
                                           pq[sl, :])

            # ================= Phase D: attention (own 2 heads) =================
            with (
                tc.tile_pool(name="s_psum", bufs=2, space="PSUM") as s_psum,
                tc.tile_pool(name="ctx_psum", bufs=3, space="PSUM") as ctx_psum,
                tc.tile_pool(name="bc_psum", bufs=1, space="PSUM") as bc_psum,
                tc.tile_pool(name="p_sbuf", bufs=4) as p_pool,
                tc.tile_pool(name="rd_sbuf", bufs=4) as rd_pool,
                tc.tile_pool(name="ctxn_sbuf", bufs=4) as ctxn_pool,
            ):
                for b in range(B):
                    for J in range(4):  # 512-query tiles (t-space)
                        qb = 4 * b + J              # global query block id
                        nt_full = 4 * J + 2         # full-width k-tiles
                        ctxA = ctx_psum.tile([65, 512], F32, name="ctxA", tag="ctx")
                        ctxB = ctx_psum.tile([65, 512], F32, name="ctxB", tag="ctx")
                        # -- full-width k-tiles --
                        for t in range(nt_full):
                            kt = kt_sb[:, b, 128 * t:128 * (t + 1)]
                            sf = s_psum.tile([128, 1024], F32, name="sh")
                            nc.tensor.matmul(sf[:, 0:512], lhsT=kt[0:64, :],
                                             rhs=qt_sb[0:64, b, 512 * J:512 * (J + 1)],
                                             tile_position=(0, 0))
                            nc.tensor.matmul(sf[:, 512:1024], lhsT=kt[64:128, :],
                                             rhs=qt_sb[64:128, b, 512 * J:512 * (J + 1)],
                                             tile_position=(64, 0))
                            pf = p_pool.tile([128, 1024], BF16, name="ph")
                            nc.scalar.activation(pf[:], sf[:], EXP, scale=1.0 / HD)
                            if t >= nt_full - 2:
                                # causal mask on q-half 0 for both heads
                                mo = 0 if t == nt_full - 2 else 256
                                nc.vector.tensor_mul(pf[:, 0:256], pf[:, 0:256],
                                                     masks_sb[:, mo:mo + 256])
                                nc.vector.tensor_mul(pf[:, 512:768], pf[:, 512:768],
                                                     masks_sb[:, mo:mo + 256])
                            nc.tensor.matmul(ctxA[:], lhsT=vaug_sb[:, 2 * b, t, :],
                                             rhs=pf[:, 0:512], start=(t == 0),
                                             stop=False, skip_group_check=True)
                            nc.tensor.matmul(ctxB[:], lhsT=vaug_sb[:, 2 * b + 1, t, :],
                                             rhs=pf[:, 512:1024], start=(t == 0),
                                             stop=False, skip_group_check=True)
                        # -- two diagonal half-tiles last (q-half 1 only) --
                        for m in range(2):
                            t = nt_full + m
                            kt = kt_sb[:, b, 128 * t:128 * (t + 1)]
                            # A in bank 0 (cols 0:256), B in bank 1 (cols 512:768):
                            # same-bank row-packed matmul pairs crash the device.
                            sh = s_psum.tile([128, 1024], F32, name="sh")
                            nc.tensor.matmul(sh[:, 0:256], lhsT=kt[0:64, :],
                                             rhs=qt_sb[0:64, b, 512 * J + 256:512 * (J + 1)],
                                             tile_position=(0, 0))
                            nc.tensor.matmul(sh[:, 512:768], lhsT=kt[64:128, :],
                                             rhs=qt_sb[64:128, b, 512 * J + 256:512 * (J + 1)],
                                             tile_position=(64, 0))
                            ph = p_pool.tile([128, 1024], BF16, name="ph")
                            moff = 512 + 512 * m  # [M0|M0] then [M1|M1]
                            nc.scalar.activation(ph[:, 0:256], sh[:, 0:256], EXP,
                                                 scale=1.0 / HD)
                            nc.scalar.activation(ph[:, 512:768], sh[:, 512:768], EXP,
                                                 scale=1.0 / HD)
                            nc.vector.tensor_mul(ph[:, 0:256], ph[:, 0:256],
                                                 masks_sb[:, moff:moff + 256])
                            nc.vector.tensor_mul(ph[:, 512:768], ph[:, 512:768],
                                                 masks_sb[:, moff:moff + 256])
                            nc.tensor.matmul(ctxA[:, 256:512],
                                             lhsT=vaug_sb[:, 2 * b, t, :],
                                             rhs=ph[:, 0:256], start=False,
                                             stop=(m == 1), skip_group_check=True)
                            nc.tensor.matmul(ctxB[:, 256:512],
                                             lhsT=vaug_sb[:, 2 * b + 1, t, :],
                                             rhs=ph[:, 512:768], start=False,
                                             stop=(m == 1), skip_group_check=True)
                        # -- normalize + evict --
                        rdA = rd_pool.tile([65, 512], F32, name="rdA")
                        rdB = rd_pool.tile([65, 512], F32, name="rdB")
                        with nc.allow_low_precision("recip"):
                            nc.vector.reciprocal(rdA[64:65, :], ctxA[64:65, :])
                            nc.vector.reciprocal(rdB[64:65, :], ctxB[64:65, :])
                        bcA = bc_psum.tile([64, 512], F32, name="bcA", tag="bc")
                        bcB = bc_psum.tile([64, 512], F32, name="bcB", tag="bc")
                        nc.tensor.matmul(bcA, lhsT=ones_sb[64:65, :],
                                         rhs=rdA[64:65, :], tile_position=(64, 0))
                        nc.tensor.matmul(bcB, lhsT=ones_sb[64:65, :],
                                         rhs=rdB[64:65, :], tile_position=(64, 0))
                        bcA_sb = rd_pool.tile([64, 512], F32, name="bcA_sb")
                        bcB_sb = rd_pool.tile([64, 512], F32, name="bcB_sb")
                        nc.vector.tensor_copy(bcA_sb, bcA[:])
                        nc.vector.tensor_copy(bcB_sb, bcB[:])
                        cnA = ctxn_pool.tile([64, 512], BF16, name="cnA")
                        cnB = ctxn_pool.tile([64, 512], BF16, name="cnB")
                        nc.vector.tensor_mul(cnA, ctxA[0:64, :], bcA_sb[:])
                        nc.vector.tensor_mul(cnB, ctxB[0:64, :], bcB_sb[:])
                        nc.gpsimd.dma_start(out=ctxn_sh[qb, 0:64, :], in_=cnA[:])
                        nc.gpsimd.dma_start(out=ctxn_sh[qb, 64:128, :], in_=cnB[:])

            # ================= AllToAll =================
            nc.gpsimd.collective_compute(
                "AllToAll",
                mybir.AluOpType.bypass,
                replica_groups=[list(range(N_CORES))],
                ins=[ctxn_sh[:].opt()],
                outs=[ctxn_a2a[:].opt()],
            )

            # ================= Phase E: output projection =================
            with (
                tc.tile_pool(name="pe_psum", bufs=4, space="PSUM") as pe_psum,
                tc.tile_pool(name="pe_sbuf", bufs=2) as pe_pool,
            ):
                ea_sb = persist.tile([128, 8, 512], BF16)
                wo_sb = persist.tile([128, 8, 1024], BF16)
                bo_sb = persist.tile([128, 1024], F32)
                nc.sync.dma_start(out=bo_sb[:], in_=bo_rep[:])
                for r in range(8):
                    nc.sync.dma_start(out=ea_sb[:, r, :], in_=ctxn_a2a[r, :, :])
                    nc.sync.dma_start(out=wo_sb[:, r, :], in_=woT[r * 128:(r + 1) * 128, :])
                for qs in range(4):
                    for dh in range(2):
                        psum_o = pe_psum.tile([128, 512], F32, name="psum_o")
                        for r in range(8):
                            nc.tensor.matmul(psum_o,
                                             lhsT=ea_sb[:, r, qs * 128:(qs + 1) * 128],
                                             rhs=wo_sb[:, r, dh * 512:(dh + 1) * 512],
                                             start=(r == 0), stop=(r == 7))
                        ot = pe_pool.tile([128, 512], F32, name="ot")
                        nc.vector.tensor_add(ot, psum_o[:], bo_sb[:, dh * 512:(dh + 1) * 512])
                        nc.sync.dma_start(
                            out=out_ext[qs * 128:(qs + 1) * 128, dh * 512:(dh + 1) * 512],
                            in_=ot[:])

    nc.compile()
    return nc


def _prep_inputs(hidden_states, Wq, Wk, Wv, Wo, bo):
    bf = ml_dtypes.bfloat16
    hs = np.asarray(hidden_states, dtype=np.float32).reshape(NT, D)
    WqT = np.ascontiguousarray(np.asarray(Wq, np.float32).T).astype(bf)
    WkT = np.ascontiguousarray(np.asarray(Wk, np.float32).T).astype(bf)
    WvT = np.ascontiguousarray(np.asarray(Wv, np.float32).T).astype(bf)
    WoT = np.ascontiguousarray(np.asarray(Wo, np.float32).T).astype(bf)
    bo_rep = np.tile(np.asarray(bo, np.float32)[None, :], (128, 1))

    p = np.arange(128)[:, None]
    f = np.arange(256)[None, :]
    M0 = (p <= f).astype(np.float32)
    M1 = (p + 128 <= f).astype(np.float32)
    masks = np.concatenate([M0, M1, M0, M0, M1, M1], axis=1).astype(bf)  # [128,1536]
    onesr = np.ones((128, 64), dtype=np.float32)
    ident = np.eye(128, dtype=np.float32).astype(bf)

    in_maps = []
    for c in range(N_CORES):
        rows = np.concatenate([np.arange(256) + b * 2048 + 256 * c for b in range(B)])
        hsT_sh = np.ascontiguousarray(hs[rows].T).astype(bf)  # [1024, 512]
        in_maps.append({
            "hsT_sh": hsT_sh,
            "wqT": WqT, "wkT": WkT, "wvT": WvT, "woT": WoT,
            "bo_rep": bo_rep, "masks": masks, "onesr": onesr, "ident": ident,
        })
    return in_maps


def kernel(hidden_states, Wq, Wk, Wv, Wo, bo, _trace=False, _trace_kwargs=None):
    if "nc" not in _cached:
        _cached["nc"] = build_nc()
    nc = _cached["nc"]
    in_maps = _prep_inputs(hidden_states, Wq, Wk, Wv, Wo, bo)
    res = run_bass_kernel_spmd(nc, in_maps, core_ids=list(range(N_CORES)),
                               trace=_trace, **(_trace_kwargs or {}))
    _cached["last_result"] = res
    out = np.concatenate([res.results[c]["out"] for c in range(N_CORES)], axis=0)
    return out.reshape(B, S, D).astype(np.float32)


# revision 11
# speedup vs baseline: 1.5082x; 1.1914x over previous
"""Distributed Trainium2 kernel for nn_Attention (causal MHA with direct-reshape
head view).

Reference semantics (B=2, S=2048, D=1024, H=16, hd=64):
    qp = hs @ Wq.T  -> [B, S, D], then q = qp.reshape(B, H, S, hd)  (DIRECT view:
    head h's token t = 16*rr + j has features qp[b, 128*h + rr, 64*j : 64*j+64])
    k, v likewise; causal attention in t with softmax(wei / hd);
    ctx -> [B, S2, H, hd] -> reshape [B, S, D]; out = ctx @ Wo.T + bo.

Sharding (uniform SPMD across 8 cores, one AllToAll):
  - Head h consumes only hs rows 128h..128h+128 (per batch). Core c owns heads
    2c, 2c+1 => hs rows 256c..256c+256 of each batch (512 rows total, disjoint
    across cores). Host passes hsT_sh = those rows, transposed [1024, 512].
  - Core projects its rows against full Wq/Wk/Wv (normal layout, to DRAM),
    then re-reads through the reshape view: token-tiles [128 t, 64 d] are flat
    8-row slices. K/Q are PE-transposed to [64 d, t] layout; V is used as-is.
  - Causal flash attention per (batch, head) in t-space; denominator rides a
    ones-column appended to V; normalization via K=1 broadcast matmul.
  - Normalized ctx^T shards [8 q-blocks, 128 dims, 512 q] go through one
    AllToAll; core c ends with all 1024 ctx dims for global query block c and
    output-projects it (+bias). Host concatenates the 8 [512, 1024] blocks.

All matmuls bf16 with fp32 PSUM accumulation; softmax needs no max-subtraction
(logits/64 are tiny).
"""

import sys

for _p in ("/opt/trn_rl_repo", "/opt/pypackages"):
    if _p not in sys.path:
        sys.path.append(_p)

import numpy as np
import ml_dtypes

import concourse.bass as bass
import concourse.mybir as mybir
import concourse.tile as tile
from concourse import bacc
from concourse.bass_utils import run_bass_kernel_spmd

N_CORES = 8
B, S, D, H = 2, 2048, 1024, 16
HD = D // H          # 64 head dim
NT = B * S           # 4096 tokens
QBLK = NT // N_CORES  # 512 query rows per output block
NROWS = 512          # hs rows owned per core (2 heads x 2 batches x 128)

BF16 = mybir.dt.bfloat16
F32 = mybir.dt.float32
F32R = mybir.dt.float32r

_cached = {}


def build_nc():
    nc = bacc.Bacc("TRN2", target_bir_lowering=False, debug=False,
                   num_devices=N_CORES)

    # ---- I/O -----------------------------------------------------------
    hsT_sh = nc.dram_tensor("hsT_sh", [D, NROWS], BF16, kind="ExternalInput")
    wqT = nc.dram_tensor("wqT", [D, D], BF16, kind="ExternalInput")
    wkT = nc.dram_tensor("wkT", [D, D], BF16, kind="ExternalInput")
    wvT = nc.dram_tensor("wvT", [D, D], BF16, kind="ExternalInput")
    woT = nc.dram_tensor("woT", [D, D], BF16, kind="ExternalInput")
    bo_rep = nc.dram_tensor("bo_rep", [128, D], F32, kind="ExternalInput")
    masks = nc.dram_tensor("masks", [128, 1536], BF16, kind="ExternalInput")
    ident = nc.dram_tensor("ident", [128, 128], BF16, kind="ExternalInput")
    sel = nc.dram_tensor("sel", [16, 1024], F32, kind="ExternalInput")
    out_ext = nc.dram_tensor("out", [QBLK, D], F32, kind="ExternalOutput")

    # internal DRAM
    qn = nc.dram_tensor("qn", [NROWS, D], BF16)   # projections, normal layout
    kn = nc.dram_tensor("kn", [NROWS, D], BF16)
    vn = nc.dram_tensor("vn", [NROWS, D], BF16)
    ctxn_sh = nc.dram_tensor("ctxn_sh", [N_CORES, 128, QBLK], BF16)
    ctxn_a2a = nc.dram_tensor("ctxn_a2a", [N_CORES, 128, QBLK], BF16)
    den_sh = nc.dram_tensor("den_sh", [N_CORES, 2, QBLK], F32)
    den_a2a = nc.dram_tensor("den_a2a", [N_CORES, 2, QBLK], F32)

    EXP = mybir.ActivationFunctionType.Exp

    with tile.TileContext(nc) as tc:
        with (
            tc.tile_pool(name="persist", bufs=1) as persist,
            tc.tile_pool(name="wtile", bufs=3) as w_pool,
            tc.tile_pool(name="evict", bufs=4) as evict_pool,
        ):
            ident_sb = persist.tile([128, 128], BF16)
            masks_sb = persist.tile([128, 1536], BF16)
            hs_sb = persist.tile([128, 8, NROWS], BF16)   # hsT_sh resident
            kt_sb = persist.tile([128, 2, 2048], BF16)    # K^T per batch (A|B rows)
            qt_sb = persist.tile([128, 2, 2048], BF16)    # Q^T per batch
            vaug_sb = persist.tile([128, 4, 16, 65], BF16)  # V tiles + ones col

            nc.sync.dma_start(out=ident_sb[:], in_=ident[:])
            nc.sync.dma_start(out=masks_sb[:], in_=masks[:])
            for dt_ in range(8):
                nc.gpsimd.dma_start(out=hs_sb[:, dt_, :],
                                    in_=hsT_sh[dt_ * 128:(dt_ + 1) * 128, :])
            nc.vector.memset(vaug_sb[:], 1.0)

            # ============ Phase A: Q/K/V projections (normal layout) ============
            with tc.tile_pool(name="pa_psum", bufs=4, space="PSUM") as pa_psum:
                for w_ext, dest in ((wqT, qn), (wkT, kn), (wvT, vn)):
                    for dh in range(2):  # dout halves of 512
                        ps = [pa_psum.tile([128, 512], F32, name="pp") for _ in range(4)]
                        for dt_ in range(8):
                            w_t = w_pool.tile([128, 512], BF16, name="w_t")
                            nc.gpsimd.dma_start(
                                out=w_t[:],
                                in_=w_ext[dt_ * 128:(dt_ + 1) * 128,
                                          dh * 512:(dh + 1) * 512])
                            for rb in range(4):
                                nc.tensor.matmul(
                                    ps[rb], lhsT=hs_sb[:, dt_, rb * 128:(rb + 1) * 128],
                                    rhs=w_t[:], start=(dt_ == 0), stop=(dt_ == 7))
                        for rb in range(4):
                            ev = evict_pool.tile([128, 512], BF16, name="ev")
                            nc.scalar.copy(ev[:], ps[rb][:])
                            nc.gpsimd.dma_start(
                                out=dest[rb * 128:(rb + 1) * 128,
                                         dh * 512:(dh + 1) * 512],
                                in_=ev[:])

            # ===== Phase B: reshape-view readback; build K^T, Q^T, V-aug =====
            with (
                tc.tile_pool(name="tr_in", bufs=4) as tr_pool,
                tc.tile_pool(name="tr_psum", bufs=2, space="PSUM") as tr_psum,
            ):
                RSH = "(a s) (j d) -> (s j) a d"
                for b in range(B):
                    for d_ in range(2):  # head delta within pair
                        rowbase = 256 * b + 128 * d_
                        nc.sync.dma_start(
                            out=vaug_sb[:, 2 * b + d_, :, 0:64],
                            in_=vn[rowbase:rowbase + 128, :].rearrange(RSH, a=16, j=16))
                        kb = tr_pool.tile([128, 16, 64], BF16, name="ktile")
                        qb_t = tr_pool.tile([128, 16, 64], BF16, name="qtile")
                        nc.sync.dma_start(
                            out=kb[:],
                            in_=kn[rowbase:rowbase + 128, :].rearrange(RSH, a=16, j=16))
                        nc.sync.dma_start(
                            out=qb_t[:],
                            in_=qn[rowbase:rowbase + 128, :].rearrange(RSH, a=16, j=16))
                        for g in range(4):  # groups of 4 token-tiles
                            pk = tr_psum.tile([128, 512], BF16, name="pk")
                            pq = tr_psum.tile([128, 512], BF16, name="pq")
                            for i in range(4):
                                m = 4 * g + i
                                tp = (0, 64 * d_)
                                nc.tensor.transpose(
                                    pk[64 * d_:64 * d_ + 64, i * 128:(i + 1) * 128],
                                    kb[:, m, :], ident_sb[:], tile_position=tp)
                                nc.tensor.transpose(
                                    pq[64 * d_:64 * d_ + 64, i * 128:(i + 1) * 128],
                                    qb_t[:, m, :], ident_sb[:], tile_position=tp)
                            sl = slice(64 * d_, 64 * d_ + 64)
                            nc.scalar.copy(kt_sb[sl, b, g * 512:(g + 1) * 512],
                                           pk[sl, :])
                            nc.scalar.copy(qt_sb[sl, b, g * 512:(g + 1) * 512],
                                           pq[sl, :])

            # ================= Phase D: attention (own 2 heads) =================
            with (
                tc.tile_pool(name="s_psum", bufs=2, space="PSUM") as s_psum,
                tc.tile_pool(name="ctx_psum", bufs=4, space="PSUM") as ctx_psum,
                tc.tile_pool(name="p_sbuf", bufs=4) as p_pool,
                tc.tile_pool(name="ctxn_sbuf", bufs=4) as ctxn_pool,
            ):
                for b in range(B):
                    for J in range(4):  # 512-query tiles (t-space)
                        qb = 4 * b + J              # global query block id
                        nt_full = 4 * J + 2         # full-width k-tiles
                        ctxA = ctx_psum.tile([65, 512], F32, name="ctxA", tag="ctx")
                        ctxB = ctx_psum.tile([65, 512], F32, name="ctxB", tag="ctx")
                        # -- full-width k-tiles --
                        for t in range(nt_full):
                            kt = kt_sb[:, b, 128 * t:128 * (t + 1)]
                            sf = s_psum.tile([128, 1024], F32, name="sh")
                            nc.tensor.matmul(sf[:, 0:512], lhsT=kt[0:64, :],
                                             rhs=qt_sb[0:64, b, 512 * J:512 * (J + 1)],
                                             tile_position=(0, 0))
                            nc.tensor.matmul(sf[:, 512:1024], lhsT=kt[64:128, :],
                                             rhs=qt_sb[64:128, b, 512 * J:512 * (J + 1)],
                                             tile_position=(64, 0))
                            pf = p_pool.tile([128, 1024], BF16, name="ph")
                            nc.scalar.activation(pf[:], sf[:], EXP, scale=1.0 / HD)
                            if t >= nt_full - 2:
                                # causal mask on q-half 0 for both heads
                                mo = 0 if t == nt_full - 2 else 256
                                nc.vector.tensor_mul(pf[:, 0:256], pf[:, 0:256],
                                                     masks_sb[:, mo:mo + 256])
                                nc.vector.tensor_mul(pf[:, 512:768], pf[:, 512:768],
                                                     masks_sb[:, mo:mo + 256])
                            nc.tensor.matmul(ctxA[:], lhsT=vaug_sb[:, 2 * b, t, :],
                                             rhs=pf[:, 0:512], start=(t == 0),
                                             stop=False, skip_group_check=True)
                            nc.tensor.matmul(ctxB[:], lhsT=vaug_sb[:, 2 * b + 1, t, :],
                                             rhs=pf[:, 512:1024], start=(t == 0),
                                             stop=False, skip_group_check=True)
                        # -- two diagonal half-tiles last (q-half 1 only) --
                        for m in range(2):
                            t = nt_full + m
                            kt = kt_sb[:, b, 128 * t:128 * (t + 1)]
                            # A in bank 0 (cols 0:256), B in bank 1 (cols 512:768):
                            # same-bank row-packed matmul pairs crash the device.
                            sh = s_psum.tile([128, 1024], F32, name="sh")
                            nc.tensor.matmul(sh[:, 0:256], lhsT=kt[0:64, :],
                                             rhs=qt_sb[0:64, b, 512 * J + 256:512 * (J + 1)],
                                             tile_position=(0, 0))
                            nc.tensor.matmul(sh[:, 512:768], lhsT=kt[64:128, :],
                                             rhs=qt_sb[64:128, b, 512 * J + 256:512 * (J + 1)],
                                             tile_position=(64, 0))
                            ph = p_pool.tile([128, 1024], BF16, name="ph")
                            moff = 512 + 512 * m  # [M0|M0] then [M1|M1]
                            nc.scalar.activation(ph[:, 0:256], sh[:, 0:256], EXP,
                                                 scale=1.0 / HD)
                            nc.scalar.activation(ph[:, 512:768], sh[:, 512:768], EXP,
                                                 scale=1.0 / HD)
                            nc.vector.tensor_mul(ph[:, 0:256], ph[:, 0:256],
                                                 masks_sb[:, moff:moff + 256])
                            nc.vector.tensor_mul(ph[:, 512:768], ph[:, 512:768],
                                                 masks_sb[:, moff:moff + 256])
                            nc.tensor.matmul(ctxA[:, 256:512],
                                             lhsT=vaug_sb[:, 2 * b, t, :],
                                             rhs=ph[:, 0:256], start=False,
                                             stop=(m == 1), skip_group_check=True)
                            nc.tensor.matmul(ctxB[:, 256:512],
                                             lhsT=vaug_sb[:, 2 * b + 1, t, :],
                                             rhs=ph[:, 512:768], start=False,
                                             stop=(m == 1), skip_group_check=True)
                        # -- evict raw ctx + den rows (normalization happens post-A2A) --
                        cnA = ctxn_pool.tile([64, 512], BF16, name="cnA")
                        cnB = ctxn_pool.tile([64, 512], BF16, name="cnB")
                        dnA = ctxn_pool.tile([1, 512], F32, name="dnA")
                        dnB = ctxn_pool.tile([1, 512], F32, name="dnB")
                        nc.vector.tensor_copy(cnA, ctxA[0:64, :])
                        nc.vector.tensor_copy(cnB, ctxB[0:64, :])
                        nc.vector.tensor_copy(dnA, ctxA[64:65, :])
                        nc.vector.tensor_copy(dnB, ctxB[64:65, :])
                        nc.gpsimd.dma_start(out=ctxn_sh[qb, 0:64, :], in_=cnA[:])
                        nc.gpsimd.dma_start(out=ctxn_sh[qb, 64:128, :], in_=cnB[:])
                        nc.gpsimd.dma_start(out=den_sh[qb, 0, :], in_=dnA[:])
                        nc.gpsimd.dma_start(out=den_sh[qb, 1, :], in_=dnB[:])

            # ================= AllToAll =================
            nc.gpsimd.collective_compute(
                "AllToAll",
                mybir.AluOpType.bypass,
                replica_groups=[list(range(N_CORES))],
                ins=[den_sh[:].opt()],
                outs=[den_a2a[:].opt()],
            )
            nc.gpsimd.collective_compute(
                "AllToAll",
                mybir.AluOpType.bypass,
                replica_groups=[list(range(N_CORES))],
                ins=[ctxn_sh[:].opt()],
                outs=[ctxn_a2a[:].opt()],
            )

            # ================= Phase E: output projection =================
            with (
                tc.tile_pool(name="pe_psum", bufs=4, space="PSUM") as pe_psum,
                tc.tile_pool(name="pe_sbuf", bufs=2) as pe_pool,
            ):
                ea_sb = persist.tile([128, 8, 512], BF16)
                ean_sb = persist.tile([128, 8, 512], BF16)
                wo_sb = persist.tile([128, 8, 1024], BF16)
                bo_sb = persist.tile([128, 1024], F32)
                sel_sb = persist.tile([16, 1024], F32)
                dall_sb = persist.tile([16, 512], F32)
                rdall_sb = persist.tile([16, 512], F32)
                nc.sync.dma_start(out=bo_sb[:], in_=bo_rep[:])
                nc.sync.dma_start(out=sel_sb[:], in_=sel[:])
                nc.sync.dma_start(out=dall_sb[:], in_=den_a2a[:].rearrange("a b q -> (a b) q"))
                with nc.allow_low_precision("den recip"):
                    nc.vector.reciprocal(rdall_sb[:], dall_sb[:])
                for r in range(8):
                    nc.sync.dma_start(out=ea_sb[:, r, :], in_=ctxn_a2a[r, :, :])
                    nc.sync.dma_start(out=wo_sb[:, r, :], in_=woT[r * 128:(r + 1) * 128, :])
                for r in range(8):
                    bc_ps = pe_psum.tile([128, 512], F32, name="bc_ps", tag="bc")
                    nc.tensor.matmul(bc_ps, lhsT=sel_sb[:, r * 128:(r + 1) * 128],
                                     rhs=rdall_sb[:], start=True, stop=True)
                    bc_sb = pe_pool.tile([128, 512], F32, name="bc_sb")
                    nc.vector.tensor_copy(bc_sb, bc_ps[:])
                    nc.vector.tensor_mul(ean_sb[:, r, :], ea_sb[:, r, :], bc_sb[:])
                for qs in range(4):
                    for dh in range(2):
                        psum_o = pe_psum.tile([128, 512], F32, name="psum_o")
                        for r in range(8):
                            nc.tensor.matmul(psum_o,
                                             lhsT=ean_sb[:, r, qs * 128:(qs + 1) * 128],
                                             rhs=wo_sb[:, r, dh * 512:(dh + 1) * 512],
                                             start=(r == 0), stop=(r == 7))
                        ot = pe_pool.tile([128, 512], F32, name="ot")
                        nc.vector.tensor_add(ot, psum_o[:], bo_sb[:, dh * 512:(dh + 1) * 512])
                        nc.sync.dma_start(
                            out=out_ext[qs * 128:(qs + 1) * 128, dh * 512:(dh + 1) * 512],
                            in_=ot[:])

    nc.compile()
    return nc


def _prep_inputs(hidden_states, Wq, Wk, Wv, Wo, bo):
    bf = ml_dtypes.bfloat16
    hs = np.asarray(hidden_states, dtype=np.float32).reshape(NT, D)
    WqT = np.ascontiguousarray(np.asarray(Wq, np.float32).T).astype(bf)
    WkT = np.ascontiguousarray(np.asarray(Wk, np.float32).T).astype(bf)
    WvT = np.ascontiguousarray(np.asarray(Wv, np.float32).T).astype(bf)
    WoT = np.ascontiguousarray(np.asarray(Wo, np.float32).T).astype(bf)
    bo_rep = np.tile(np.asarray(bo, np.float32)[None, :], (128, 1))

    p = np.arange(128)[:, None]
    f = np.arange(256)[None, :]
    M0 = (p <= f).astype(np.float32)
    M1 = (p + 128 <= f).astype(np.float32)
    masks = np.concatenate([M0, M1, M0, M0, M1, M1], axis=1).astype(bf)  # [128,1536]
    ident = np.eye(128, dtype=np.float32).astype(bf)
    sel = np.zeros((16, 1024), dtype=np.float32)
    for r in range(N_CORES):
        for m in range(128):
            sel[2 * r + m // 64, 128 * r + m] = 1.0

    in_maps = []
    for c in range(N_CORES):
        rows = np.concatenate([np.arange(256) + b * 2048 + 256 * c for b in range(B)])
        hsT_sh = np.ascontiguousarray(hs[rows].T).astype(bf)  # [1024, 512]
        in_maps.append({
            "hsT_sh": hsT_sh,
            "wqT": WqT, "wkT": WkT, "wvT": WvT, "woT": WoT,
            "bo_rep": bo_rep, "masks": masks, "ident": ident, "sel": sel,
        })
    return in_maps


def kernel(hidden_states, Wq, Wk, Wv, Wo, bo, _trace=False, _trace_kwargs=None):
    if "nc" not in _cached:
        _cached["nc"] = build_nc()
    nc = _cached["nc"]
    in_maps = _prep_inputs(hidden_states, Wq, Wk, Wv, Wo, bo)
    res = run_bass_kernel_spmd(nc, in_maps, core_ids=list(range(N_CORES)),
                               trace=_trace, **(_trace_kwargs or {}))
    _cached["last_result"] = res
    out = np.concatenate([res.results[c]["out"] for c in range(N_CORES)], axis=0)
    return out.reshape(B, S, D).astype(np.float32)
